# revision 2
# baseline (speedup 1.0000x reference)
"""HadLinear TRN2 kernel v5: out = fwht_1024blocks(x)/sqrt(1024) @ W.T

Host stages per-core x^T and shared W^T as bf16 (sharding/layout prep, like
the H constant) so every device read is a plain contiguous-chunk DMA — no
fp32->bf16 prep pass, no XBAR transpose instructions (v4's early 360us was
prep-DMA supply-limited; v2's 640 XBAR transposes cost 1.25us each).

Per core (row-shard of x, full W):
  Phase A: per (strip, 1024-block): xbig[p,j,m] <- x^T slice; butterfly
           stage 1 ON THE PE via +/-H accumulation into PSUM pairs
           (PE idles early anyway; halves the DVE chain gating early B):
             s1[j]   = H@V_j + H@V_{j+4}   (2 accumulating matmuls)
             s1[j+4] = H@V_j - H@V_{j+4}   (lhsT = -H for the second)
           stages 2/3 on DVE (stage-2 PSUM bounce copies on idle ACT),
           bf16 outputs = phase-B lhsT tiles.
  Phase B: psum[m,n] += A[kt]-slice.T @ W^T[kt] over 32 k-tiles; evict to
           bf16 (DVE), store on the ACT HWDGE queue; host upcasts y.

Self-contained: hardcodes shapes B=4, S=4096, D_in=D_out=4096, 8 cores.
"""

import numpy as np
import ml_dtypes

import concourse.bacc as bacc
import concourse.mybir as mybir
import concourse.tile as tile
from concourse.bass_utils import run_bass_kernel_spmd

P = 128
N_CORES = 8
B_FULL, S_FULL, D = 4, 4096, 4096
M_FULL = B_FULL * S_FULL          # 16384 rows total
M_CORE = M_FULL // N_CORES        # 2048 rows per core
HAD = 1024                        # hadamard block
STRIP = 512                       # row-strip width (phase A moving free dim)
NSTRIP = 512                      # out-feature strip width


def _h128_np():
    """H_128 (natural order) scaled by 1/sqrt(1024) = 2^-5; exact in bf16."""
    h = np.array([[(-1.0) ** bin(i & j).count("1") for j in range(P)]
                  for i in range(P)])
    return (h / 32.0).astype(ml_dtypes.bfloat16)


def build_nc(m_core=M_CORE, d=D, n_out=D, halves=2):
    """Build the per-core Bass kernel (SPMD: same program all cores)."""
    f32, bf16 = mybir.dt.float32, mybir.dt.bfloat16
    nc = bacc.Bacc(None, target_bir_lowering=False, debug=False)

    xt = nc.declare_dram_parameter("xt", [d, m_core], bf16, isOutput=False)
    wt = nc.declare_dram_parameter("wt", [d, n_out], bf16, isOutput=False)
    h = nc.declare_dram_parameter("h", [P, P], bf16, isOutput=False)
    hn = nc.declare_dram_parameter("hn", [P, P], bf16, isOutput=False)
    y = nc.declare_dram_parameter("y", [m_core, n_out], bf16, isOutput=True)

    n_blk = d // HAD
    kt_total = d // P                 # 32 k-tiles
    ms_total = m_core // STRIP        # row strips (4)
    ms_per_half = ms_total // halves
    msub_per_half = (m_core // halves) // P   # 8 output-row tiles per half
    ns_total = n_out // NSTRIP        # 8 out strips

    with tile.TileContext(nc) as tc:
        with (
            tc.tile_pool(name="const", bufs=1) as constp,
            tc.tile_pool(name="apool", bufs=(kt_total * ms_per_half * 5) // 4) as apool,
            tc.tile_pool(name="xbf", bufs=3) as xbfp,
            tc.tile_pool(name="ev", bufs=6) as evp,
            tc.tile_pool(name="bfly", bufs=14) as bflyp,
            tc.tile_pool(name="wbf", bufs=2) as wbfp,
            tc.tile_pool(name="outp", bufs=6) as outp,
            tc.tile_pool(name="ps", bufs=8, space="PSUM") as psp,
        ):
            h128 = constp.tile([P, P], bf16, tag="h", name="h128")
            nc.scalar.dma_start(out=h128[:], in_=h[:])
            hn128 = constp.tile([P, P], bf16, tag="hn", name="hn128")
            nc.scalar.dma_start(out=hn128[:], in_=hn[:])

            for half in range(halves):
                a_tiles = {}
                # ---- Phase A: FWHT of this half's row strips ----
                for msl in range(ms_per_half):
                    ms = half * ms_per_half + msl
                    m0 = ms * STRIP
                    for blk in range(n_blk):
                        # xbig[p, j, m] = x^T[blk*1024 + j*128 + p, m0+m]
                        xbig = xbfp.tile([P, HAD // P, STRIP], bf16,
                                         tag="xbf", name=f"xbig_{ms}_{blk}")
                        src = xt[blk * HAD:(blk + 1) * HAD, m0:m0 + STRIP]
                        nc.sync.dma_start(
                            out=xbig[:],
                            in_=src.rearrange("(j p) m -> p j m", p=P))
                        # stage 1 on PE: per pair-group jg, j in {jg, jg+2}:
                        #   s1[j]   = H V_j + H V_{j+4}
                        #   s1[j+4] = H V_j - H V_{j+4}
                        # then stage 2 immediately (frees the 4 PSUM banks)
                        s2 = [None] * 8
                        for jg in range(2):
                            s1 = {}
                            for j in (jg, jg + 2):
                                vs = psp.tile([P, STRIP], f32, tag="ps",
                                              name=f"s1s_{ms}_{blk}_{j}")
                                nc.tensor.matmul(vs[:], lhsT=h128[:],
                                                 rhs=xbig[:, j, :],
                                                 start=True, stop=False)
                                nc.tensor.matmul(vs[:], lhsT=h128[:],
                                                 rhs=xbig[:, j + 4, :],
                                                 start=False, stop=True)
                                vd = psp.tile([P, STRIP], f32, tag="ps",
                                              name=f"s1d_{ms}_{blk}_{j}")
                                nc.tensor.matmul(vd[:], lhsT=h128[:],
                                                 rhs=xbig[:, j, :],
                                                 start=True, stop=False)
                                nc.tensor.matmul(vd[:], lhsT=hn128[:],
                                                 rhs=xbig[:, j + 4, :],
                                                 start=False, stop=True)
                                s1[j], s1[j + 4] = vs, vd
                            # stage 2 pairs (jg, jg+2) and (jg+4, jg+6);
                            # DVE reads ONE PSUM input: bounce the left
                            # element via ACT (idle early) to SBUF.
                            for j in (jg, jg + 4):
                                e = evp.tile([P, STRIP], f32, tag="ev",
                                             name=f"e_{ms}_{blk}_{j}")
                                nc.scalar.copy(out=e[:], in_=s1[j][:])
                                s2[j] = bflyp.tile([P, STRIP], bf16, tag="bfly",
                                                   name=f"s2_{ms}_{blk}_{j}")
                                s2[j + 2] = bflyp.tile([P, STRIP], bf16, tag="bfly",
                                                       name=f"s2_{ms}_{blk}_{j+2}")
                                nc.vector.tensor_add(out=s2[j][:], in0=e[:],
                                                     in1=s1[j + 2][:])
                                nc.vector.tensor_sub(out=s2[j + 2][:], in0=e[:],
                                                     in1=s1[j + 2][:])
                        # stage 3: all-SBUF bf16 (DVE 2x packed)
                        for j in (0, 2, 4, 6):
                            kt_a, kt_b = blk * 8 + j, blk * 8 + j + 1
                            ta = apool.tile([P, STRIP], bf16, tag="A",
                                            name=f"A_{half}_{kt_a}_{msl}")
                            tb = apool.tile([P, STRIP], bf16, tag="A",
                                            name=f"A_{half}_{kt_b}_{msl}")
                            nc.vector.tensor_add(out=ta[:], in0=s2[j][:], in1=s2[j + 1][:])
                            nc.vector.tensor_sub(out=tb[:], in0=s2[j][:], in1=s2[j + 1][:])
                            a_tiles[(kt_a, msl)] = ta
                            a_tiles[(kt_b, msl)] = tb

                # ---- Phase B: C tiles = A-slice.T @ W^T tile, contiguous out ----
                for ns in range(ns_total):
                    n0 = ns * NSTRIP
                    # wbig[p, kt, n] = W^T[kt*128 + p, n0+n]; contiguous 1KB
                    # row chunks, one 4MB DMA per n-strip
                    wbig = wbfp.tile([P, kt_total, NSTRIP], bf16,
                                     tag="wbf", name=f"wbig_{half}_{ns}")
                    nc.sync.dma_start(
                        out=wbig[:],
                        in_=wt[:, n0:n0 + NSTRIP].rearrange(
                            "(kt p) n -> p kt n", p=P))
                    for gr in range(msub_per_half // 4):
                        subs = [gr * 4 + i for i in range(4)]
                        cps = {g: psp.tile([P, NSTRIP], f32, tag="ps",
                                           name=f"c_{half}_{ns}_{g}")
                               for g in subs}
                        for kt in range(kt_total):
                            for g in subs:
                                msl, sub = divmod(g, STRIP // P)
                                nc.tensor.matmul(
                                    cps[g][:],
                                    lhsT=a_tiles[(kt, msl)][:, sub * P:(sub + 1) * P],
                                    rhs=wbig[:, kt, :],
                                    start=(kt == 0), stop=(kt == kt_total - 1),
                                )
                        for g in subs:
                            r0 = (half * msub_per_half + g) * P
                            cout = outp.tile([P, NSTRIP], bf16, tag="outp",
                                             name=f"co_{half}_{ns}_{g}")
                            nc.vector.tensor_copy(out=cout[:], in_=cps[g][:])
                            nc.scalar.dma_start(out=y[r0:r0 + P, n0:n0 + NSTRIP],
                                                in_=cout[:])
    nc.compile()
    return nc


_CACHE = {}


def _get_nc():
    if "nc" not in _CACHE:
        _CACHE["nc"] = build_nc()
    return _CACHE["nc"]


def run(x, weight, trace=False):
    assert x.shape == (B_FULL, S_FULL, D) and weight.shape == (D, D)
    nc = _get_nc()
    xf = np.asarray(x, dtype=np.float32).reshape(M_FULL, D)
    # host staging (sharding/layout): bf16 + transpose so device reads are
    # contiguous; the FWHT result is bit-identical to casting on device
    wtb = np.ascontiguousarray(
        np.asarray(weight, dtype=np.float32).astype(ml_dtypes.bfloat16).T)
    hh = _h128_np()
    hhn = (-hh).astype(ml_dtypes.bfloat16)
    in_maps = []
    for c in range(N_CORES):
        xtb = np.ascontiguousarray(
            xf[c * M_CORE:(c + 1) * M_CORE].astype(ml_dtypes.bfloat16).T)
        in_maps.append({"xt": xtb, "wt": wtb, "h": hh, "hn": hhn})
    res = run_bass_kernel_spmd(nc, in_maps, core_ids=list(range(N_CORES)),
                               trace=trace)
    yv = np.concatenate([r["y"] for r in res.results], axis=0)
    return yv.astype(np.float32).reshape(B_FULL, S_FULL, D), res


def kernel(x, weight):
    return run(x, weight)[0]
